# revision 1
# baseline (speedup 1.0000x reference)
"""Chamfer loss (complex Minkowski metric) Trainium2 Bass kernel.

Full inputs p, q: (2, 64, 512, 4) fp32.  Output: scalar fp32.

Math: for each (b, n, m):
  m_real = <d_re, d_re>_L,  m_im = 2 <d_re, d_im>_L   (L = diag(1,-1,-1,-1))
  dist   = sqrt(m_real^2 + m_im^2 + eps)
  loss   = sum_bn min_m dist + sum_bm min_n dist

Expanding the bilinear forms:
  m_real[n,m] = A[n] - 2<p_re[n], q_re[m]>_L + C[m]
  m_im[n,m]   = 2 h_p[n] - 2<p_re[n], q_im[m]>_L - 2<p_im[n], q_re[m]>_L + 2 h_q[m]
Both are single matmuls with the per-point constants folded in as extra
rank-1 rows: K=12 for m_real, K=16 for m_im.

Device layout per core (8 batches): one K-stacked weight tile per side,
one (128,1024) PSUM tile per n-chunk holding [m_real | m_im], ACT square,
DVE tensor_tensor_reduce fusing (sq_r + sq_i) with the min-over-m,
elementwise min across the 4 n-chunks, PE-transpose + free-dim reduce for
the min-over-n, then sqrt/sum epilogue to one scalar per core.

Sharding: pure data parallel over batch (8 batches per core); host sums
the 8 per-core partial scalars.
"""

import os

import numpy as np

import concourse.bacc as bacc
import concourse.bass as bass
import concourse.mybir as mybir
import concourse.tile as tile
from concourse.bass_utils import run_bass_kernel_spmd

AluOp = mybir.AluOpType
F32R = mybir.dt.float32r
ACT = mybir.ActivationFunctionType
F32 = mybir.dt.float32
BF16 = mybir.dt.bfloat16

N_CORES = 8
B_PER_CORE = 8
NPTS = 512          # N == M == 512
NCHUNK = 4          # 512 / 128
BIG = 3.0e38
EPS = 1e-12


def build_nc(v_dtype=F32, reps=1):
    """Build the per-core SPMD Bass module.

    reps > 1 repeats the whole compute loop (same data, idempotent
    results) so wall-clock slope over reps isolates per-iteration device
    time from dispatch overhead.
    """
    nc = bacc.Bacc("TRN2", target_bir_lowering=False, debug=False)

    raw_d = nc.dram_tensor("raw", [64, 2 * NPTS], F32, kind="ExternalInput")
    ident_d = nc.dram_tensor("ident", [128, 128], v_dtype, kind="ExternalInput")
    svec_d = nc.dram_tensor("svec", [128, 4], F32, kind="ExternalInput")
    out_d = nc.dram_tensor("out", [1, 1], F32, kind="ExternalOutput")

    with tile.TileContext(nc) as tc:
        with (
            tc.tile_pool(name="const", bufs=1) as constp,
            tc.tile_pool(name="prep", bufs=1) as prep,
            tc.tile_pool(name="wts", bufs=1) as wp,
            tc.tile_pool(name="sq", bufs=6) as sqp,
            tc.tile_pool(name="v", bufs=3) as vp,
            tc.tile_pool(name="red", bufs=3) as redp,
            tc.tile_pool(name="fin", bufs=1) as finp,
            tc.tile_pool(name="ps", bufs=3, space=bass.MemorySpace.PSUM) as psp,
            tc.tile_pool(name="pst", bufs=2, space=bass.MemorySpace.PSUM) as pstp,
        ):
            # ---------------- input + constants ----------------
            # raw rows: [p(32) | q(32)], row within block = 4*b + k;
            # free cols [0:512] = real part, [512:1024] = imag part.
            # (re/im share partitions so tensor_tensor ops satisfy the
            # same-base-partition rule; they differ only in free offset.)
            staging = prep.tile([64, 2 * NPTS], F32, tag="staging")
            nc.sync.dma_start(staging[:], raw_d[:])
            svec = constp.tile([128, 4], F32, tag="svec")
            nc.sync.dma_start(svec[:], svec_d[:])
            ident = constp.tile([128, 128], v_dtype, tag="ident")
            nc.scalar.dma_start(ident[:], ident_d[:])
            # svec cols: 0 = lam tiled, 1 = 2*lam tiled, 2 = -2*lam tiled
            eps_t = constp.tile([128, 1], F32, tag="eps")
            nc.gpsimd.memset(eps_t[:], EPS)

            # ---------------- prep: squares / cross products / scaling ----
            sq_t = prep.tile([64, NPTS], F32, tag="sq_t")  # re^2 (p rows, q rows)
            nc.vector.tensor_mul(sq_t[:], staging[:, 0:NPTS], staging[:, 0:NPTS])
            sq_s = prep.tile([64, NPTS], F32, tag="sq_s")  # lam * re^2
            nc.vector.tensor_scalar_mul(sq_s[:], sq_t[:], svec[0:64, 0:1])

            cross_t = prep.tile([64, NPTS], F32, tag="cross_t")  # re*im
            nc.vector.tensor_mul(cross_t[:], staging[:, 0:NPTS], staging[:, NPTS : 2 * NPTS])
            cross_s = prep.tile([64, NPTS], F32, tag="cross_s")  # 2*lam * re*im
            nc.vector.tensor_scalar_mul(cross_s[:], cross_t[:], svec[0:64, 1:2])

            p_s = prep.tile([32, 2 * NPTS], F32, tag="p_s")  # -2*lam * p_{re|im}
            nc.vector.tensor_scalar_mul(p_s[:], staging[0:32, :], svec[0:32, 2:3])

            # ---------------- assembled K-stacked matmul operands ---------
            # per-batch 12-row block: lhsT_real=[lam*p_re^2; -2lam*p_re; ones]
            #                         rhs_real =[ones; q_re; lam*q_re^2]
            # per-batch 16-row block: lhsT_im=[2lam*p_re*p_im; -2lam*p_re;
            #                                  -2lam*p_im; ones]
            #                         rhs_im =[ones; q_im; q_re; 2lam*q_re*q_im]
            # Matmul base partition must be in {0,32,64}: 4 tiles per kind,
            # each (64,512) holding 2 batches at bases 0 and 32.
            ltr_g, rtr_g, lti_g, rti_g = [], [], [], []
            for g in range(4):
                ltr_g.append(wp.tile([64, NPTS], F32, tag=f"ltr{g}", name=f"ltr{g}"))
                rtr_g.append(wp.tile([64, NPTS], F32, tag=f"rtr{g}", name=f"rtr{g}"))
                lti_g.append(wp.tile([64, NPTS], F32, tag=f"lti{g}", name=f"lti{g}"))
                rti_g.append(wp.tile([64, NPTS], F32, tag=f"rti{g}", name=f"rti{g}"))
            # emit fills in batch order so batch 0 is runnable after its own
            # 10 scatters rather than the whole group's 20
            for b in range(B_PER_CORE):
                g, j = b // 2, b % 2
                o = 32 * j
                ltr, rtr = ltr_g[g], rtr_g[g]
                lti, rti = lti_g[g], rti_g[g]
                if j == 0:  # ones rows + padding, whole group tile at once
                    nc.gpsimd.memset(ltr[:], 1.0)
                    nc.gpsimd.memset(rtr[:], 1.0)
                    nc.gpsimd.memset(lti[:], 1.0)
                    nc.gpsimd.memset(rti[:], 1.0)
                rp = slice(4 * b, 4 * b + 4)        # p rows of batch b
                rq = slice(32 + 4 * b, 32 + 4 * b + 4)  # q rows of batch b
                RE = slice(0, NPTS)
                IM = slice(NPTS, 2 * NPTS)
                moves = [
                    (ltr[o : o + 4, :], sq_s[rp, :]),
                    (ltr[o + 4 : o + 8, :], p_s[rp, RE]),
                    (rtr[o + 4 : o + 8, :], staging[rq, RE]),
                    (rtr[o + 8 : o + 12, :], sq_s[rq, :]),
                    (lti[o : o + 4, :], cross_s[rp, :]),
                    (lti[o + 4 : o + 8, :], p_s[rp, RE]),
                    (lti[o + 8 : o + 12, :], p_s[rp, IM]),
                    (rti[o + 4 : o + 8, :], staging[rq, IM]),
                    (rti[o + 8 : o + 12, :], staging[rq, RE]),
                    (rti[o + 12 : o + 16, :], cross_s[rq, :]),
                ]
                for dst, srcv in moves:
                    nc.sync.dma_start(dst, srcv)

            # mins cols 0:32 -> min over m (one per n-chunk, per batch)
            # mins cols 32:64 -> min over n (4 m-blocks per batch)
            macc = finp.tile([128, 64], F32, tag="macc")
            nc.gpsimd.memset(macc[:], BIG)

            for rep in range(reps):
              mins = finp.tile([128, 64], F32, tag="mins", bufs=2, name=f"mins{rep}")
              for b in range(B_PER_CORE):
                  g, j = b // 2, b % 2
                  base = 32 * j
                  # all 4 n-chunks of this batch share one (128,2048) v tile
                  vbat = vp.tile([128, NCHUNK * NPTS], v_dtype, tag="vbat")
                  for c in range(NCHUNK):
                      pp = psp.tile([128, 2 * NPTS], F32, tag="pp")
                      nc.tensor.matmul(
                          pp[:, 0:NPTS],
                          ltr_g[g][base : base + 12, bass.ts(c, 128)].bitcast(F32R),
                          rtr_g[g][base : base + 12, :].bitcast(F32R),
                      )
                      nc.tensor.matmul(
                          pp[:, NPTS : 2 * NPTS],
                          lti_g[g][base : base + 16, bass.ts(c, 128)].bitcast(F32R),
                          rti_g[g][base : base + 16, :].bitcast(F32R),
                      )
                      sq = sqp.tile([128, 2 * NPTS], v_dtype, tag="sq")
                      nc.scalar.activation(sq[:], pp[:], ACT.Square)
                      mode = os.environ.get("KERNEL_ADDS", "pool1")
                      if mode == "dve":
                          add_eng = nc.vector
                      elif mode == "pool2":
                          add_eng = nc.gpsimd if c in (0, 2) else nc.vector
                      else:
                          add_eng = nc.gpsimd if c == 0 else nc.vector
                      add_eng.tensor_add(
                          vbat[:, bass.ts(c, NPTS)],
                          sq[:, 0:NPTS],
                          sq[:, NPTS : 2 * NPTS],
                      )
                  # rowmin in two halves so the first can start after chunk 1
                  v3a = vbat[:, 0 : 2 * NPTS].rearrange("p (c m) -> p c m", c=2)
                  v3b = vbat[:, 2 * NPTS : 4 * NPTS].rearrange("p (c m) -> p c m", c=2)
                  nc.vector.tensor_reduce(
                      mins[:, 4 * b : 4 * b + 2], v3a,
                      axis=mybir.AxisListType.X, op=AluOp.min,
                  )
                  # cross-chunk elementwise min as a chain: each step right
                  # after its chunk's add
                  m01 = redp.tile([128, NPTS], v_dtype, tag="m01")
                  nc.vector.tensor_tensor(
                      m01[:], vbat[:, 0:NPTS], vbat[:, NPTS : 2 * NPTS], op=AluOp.min
                  )
                  m012 = redp.tile([128, NPTS], v_dtype, tag="m012")
                  nc.vector.tensor_tensor(
                      m012[:], m01[:], vbat[:, 2 * NPTS : 3 * NPTS], op=AluOp.min
                  )
                  nc.vector.tensor_reduce(
                      mins[:, 4 * b + 2 : 4 * b + 4], v3b,
                      axis=mybir.AxisListType.X, op=AluOp.min,
                  )
                  acc = redp.tile([128, NPTS], v_dtype, tag="acc")
                  nc.vector.tensor_tensor(
                      acc[:], m012[:], vbat[:, 3 * NPTS : 4 * NPTS], op=AluOp.min
                  )

                  pt = pstp.tile([128, NPTS], v_dtype, tag="pt", bufs=1)
                  tdt = F32R if v_dtype == F32 else v_dtype
                  for c in range(NCHUNK):
                      nc.tensor.transpose(
                          pt[:, bass.ts(c, 128)].bitcast(tdt),
                          acc[:, bass.ts(c, 128)].bitcast(tdt),
                          ident[:].bitcast(tdt),
                      )
                  pt3 = pt[:].rearrange("p (c f) -> p c f", c=NCHUNK)
                  nc.vector.tensor_reduce(
                      mins[:, 32 + 4 * b : 32 + 4 * b + 4],
                      pt3,
                      axis=mybir.AxisListType.X,
                      op=AluOp.min,
                  )

              nc.vector.tensor_tensor(macc[:], macc[:], mins[:], op=AluOp.min)

            # ---------------- epilogue: sqrt, sum all ---------------------
            smins = finp.tile([128, 64], F32, tag="smins")
            nc.scalar.activation(smins[:], macc[:], ACT.Sqrt, bias=eps_t[:])
            colsum = finp.tile([128, 1], F32, tag="colsum")
            nc.vector.tensor_reduce(
                colsum[:], smins[:], axis=mybir.AxisListType.X, op=AluOp.add
            )
            ones_t = constp.tile([128, 1], F32, tag="ones")
            nc.gpsimd.memset(ones_t[:], 1.0)
            ps_out = pstp.tile([1, 1], F32, tag="ps_out", bufs=1)
            nc.tensor.matmul(ps_out[:], ones_t[:], colsum[:])
            out_sb = finp.tile([1, 1], F32, tag="out_sb")
            nc.vector.tensor_copy(out_sb[:], ps_out[:])
            nc.sync.dma_start(out_d[:], out_sb[:])

    nc.compile()
    return nc


_NC_CACHE = {}


def _get_nc():
    key = os.environ.get("KERNEL_VDT", "bf16")
    if key not in _NC_CACHE:
        _NC_CACHE[key] = build_nc(BF16 if key == "bf16" else F32)
    return _NC_CACHE[key]


def make_in_maps(p, q, v_dtype_key="f32"):
    """Host-side shard + layout: pure reshuffling plus constant injection."""
    p = np.ascontiguousarray(np.asarray(p, dtype=np.float32))
    q = np.ascontiguousarray(np.asarray(q, dtype=np.float32))
    lam = np.array([1.0, -1.0, -1.0, -1.0], dtype=np.float32)
    svec = np.zeros((128, 4), dtype=np.float32)
    svec[:, 0] = np.tile(lam, 32)
    svec[:, 1] = np.tile(2.0 * lam, 32)
    svec[:, 2] = np.tile(-2.0 * lam, 32)
    if v_dtype_key == "bf16":
        import ml_dtypes

        ident = np.eye(128, dtype=ml_dtypes.bfloat16)
    else:
        ident = np.eye(128, dtype=np.float32)
    in_maps = []
    for c in range(N_CORES):
        sl = slice(c * B_PER_CORE, (c + 1) * B_PER_CORE)
        # (2,8,512,4) -> per part (8,4,512) -> (32,512); row = 4*b + k.
        # raw: rows [p(32) | q(32)], free cols [re(512) | im(512)].
        pre = np.ascontiguousarray(p[0, sl].transpose(0, 2, 1)).reshape(32, NPTS)
        pim = np.ascontiguousarray(p[1, sl].transpose(0, 2, 1)).reshape(32, NPTS)
        qre = np.ascontiguousarray(q[0, sl].transpose(0, 2, 1)).reshape(32, NPTS)
        qim = np.ascontiguousarray(q[1, sl].transpose(0, 2, 1)).reshape(32, NPTS)
        raw = np.concatenate(
            [
                np.concatenate([pre, pim], axis=1),
                np.concatenate([qre, qim], axis=1),
            ],
            axis=0,
        )
        in_maps.append({"raw": raw, "ident": ident, "svec": svec})
    return in_maps


def kernel(p, q):
    nc = _get_nc()
    in_maps = make_in_maps(p, q, os.environ.get("KERNEL_VDT", "bf16"))
    res = run_bass_kernel_spmd(nc, in_maps, core_ids=list(range(N_CORES)))
    total = np.float64(0.0)
    for r in res.results:
        total += np.float64(r["out"][0, 0])
    return np.asarray(total, dtype=np.float32).reshape(())



# revision 6
# speedup vs baseline: 1.2607x; 1.2607x over previous
"""Chamfer loss (complex Minkowski metric) Trainium2 Bass kernel.

Full inputs p, q: (2, 64, 512, 4) fp32.  Output: scalar fp32.

Math: for each (b, n, m):
  m_real = <d_re, d_re>_L,  m_im = 2 <d_re, d_im>_L   (L = diag(1,-1,-1,-1))
  dist   = sqrt(m_real^2 + m_im^2 + eps)
  loss   = sum_bn min_m dist + sum_bm min_n dist

Key idea: sq = m_real^2 + m_im^2 is a bilinear form of rank 76 in
quartic monomial features of the two point sets, so one matmul per
(batch, n-chunk) yields sq directly in PSUM -- no ACT Square, no DVE
add.  PE f32r truncates inputs to 11 mantissa bits, which is far too
coarse for the tiny minima, so each feature f is Dekker-split:
  K_f L_f = Kh Lh + Kh Llo + Klo Lh   (+ O(2^-24) dropped)
giving 76 + 152 = 228 rank-1 terms.  Two PSUM-accumulating matmuls per
chunk: MM1 f32r K=128 (the 76 hi*hi terms + 52 largest corrections,
all pre-rounded to 11 bits so hardware truncation is a no-op), MM2
bf16 K=100 (remaining corrections; they are 2^-12-scale, so bf16 is
plenty).  Measured end-to-end rel err ~1e-4.

Device per batch: 8 matmuls -> PSUM f32; ACT copies PSUM -> SBUF bf16
(the only full-matrix evacuation); DVE reduce = min over m; DVE
elementwise-min chain over the 4 n-chunks; PE transpose + DVE reduce =
min over n; sqrt/sum epilogue to one scalar per core.

Sharding: pure data parallel over batch (8 per core); host sums the 8
per-core partials.
"""

import os

import numpy as np

import concourse.bacc as bacc
import concourse.bass as bass
import concourse.mybir as mybir
import concourse.tile as tile
from concourse.bass_utils import run_bass_kernel_spmd

AluOp = mybir.AluOpType
F32R = mybir.dt.float32r
ACT = mybir.ActivationFunctionType
F32 = mybir.dt.float32
BF16 = mybir.dt.bfloat16

N_CORES = 8
B_PER_CORE = 8
NPTS = 512
NCHUNK = 4
K1 = 128            # MM1 rows: 76 hi*hi + 52 largest corrections (f32r)
K2 = 100            # MM2 rows: remaining Dekker corrections (bf16)
BIG = 3.0e38
EPS = 1e-12


def build_nc(v_dtype=BF16, reps=1):
    """Build the per-core SPMD Bass module.

    reps > 1 repeats the whole compute loop (same data, idempotent
    results) so wall-clock slope over reps isolates per-iteration device
    time from dispatch overhead.
    """
    nc = bacc.Bacc("TRN2", target_bir_lowering=False, debug=False)

    NW = B_PER_CORE * NPTS
    kf1_d = nc.dram_tensor("kf1", [K1, NW], F32, kind="ExternalInput")
    lf1_d = nc.dram_tensor("lf1", [K1, NW], F32, kind="ExternalInput")
    kf2_d = nc.dram_tensor("kf2", [K2, NW], BF16, kind="ExternalInput")
    lf2_d = nc.dram_tensor("lf2", [K2, NW], BF16, kind="ExternalInput")
    ident_d = nc.dram_tensor("ident", [128, 128], BF16, kind="ExternalInput")
    out_d = nc.dram_tensor("out", [1, 1], F32, kind="ExternalOutput")

    with tile.TileContext(nc) as tc:
        with (
            tc.tile_pool(name="const", bufs=1) as constp,
            tc.tile_pool(name="prep", bufs=1) as prep,
            tc.tile_pool(name="wts", bufs=1) as wp,
            tc.tile_pool(name="sq", bufs=3) as sqp,
            tc.tile_pool(name="red", bufs=2) as redp,
            tc.tile_pool(name="fin", bufs=1) as finp,
            tc.tile_pool(name="ps", bufs=3, space=bass.MemorySpace.PSUM) as psp,
            tc.tile_pool(name="pst", bufs=1, space=bass.MemorySpace.PSUM) as pstp,
        ):
            # -------- one-time input staging (outside the rep loop) -----
            # DMA hop DRAM -> staging -> operand tile: the BIR verifier
            # requires f32r matmul operands to come from an SBUF source.
            stg_k = prep.tile([K1, NW], F32, tag="stg_k")
            stg_l = prep.tile([K1, NW], F32, tag="stg_l")
            nc.sync.dma_start(stg_k[:], kf1_d[:])
            nc.sync.dma_start(stg_l[:], lf1_d[:])
            kf1 = wp.tile([K1, NW], F32, tag="kf1")
            lf1 = wp.tile([K1, NW], F32, tag="lf1")
            nc.sync.dma_start(kf1[:], stg_k[:])
            nc.sync.dma_start(lf1[:], stg_l[:])
            kf2 = wp.tile([K2, NW], BF16, tag="kf2")
            lf2 = wp.tile([K2, NW], BF16, tag="lf2")
            nc.sync.dma_start(kf2[:], kf2_d[:])
            nc.sync.dma_start(lf2[:], lf2_d[:])
            ident = constp.tile([128, 128], BF16, tag="ident")
            nc.scalar.dma_start(ident[:], ident_d[:])
            eps_t = constp.tile([128, 1], F32, tag="eps")
            nc.gpsimd.memset(eps_t[:], EPS)
            ones_t = constp.tile([128, 1], F32, tag="ones")
            nc.gpsimd.memset(ones_t[:], 1.0)

            # mins cols 0:32  -> min over m (4 n-chunks per batch)
            # mins cols 32:64 -> min over n (4 m-blocks per batch)
            macc = finp.tile([128, 64], BF16, tag="macc")
            nc.gpsimd.memset(macc[:], BIG)

            for rep in range(reps):
                mins = finp.tile([128, 64], BF16, tag="mins", bufs=2,
                                 name=f"mins{rep}")
                for b in range(B_PER_CORE):
                    ob = b * NPTS
                    sqb = sqp.tile([128, NCHUNK * NPTS], BF16, tag="sqb")
                    m01 = redp.tile([128, NPTS], BF16, tag="m01")
                    m23 = redp.tile([128, NPTS], BF16, tag="m23")
                    acc = redp.tile([128, NPTS], BF16, tag="acc")
                    for h in range(2):
                        psq = psp.tile([128, 2 * NPTS], F32, tag="psq")
                        for j in range(2):
                            c = 2 * h + j
                            cs = slice(ob + 128 * c, ob + 128 * c + 128)
                            nc.tensor.matmul(
                                psq[:, bass.ts(j, NPTS)],
                                kf1[:, cs].bitcast(F32R),
                                lf1[:, ob : ob + NPTS].bitcast(F32R),
                                start=True, stop=False,
                            )
                            nc.tensor.matmul(
                                psq[:, bass.ts(j, NPTS)],
                                kf2[:, cs],
                                lf2[:, ob : ob + NPTS],
                                start=False, stop=True,
                            )
                        # the ONLY full-matrix PSUM evacuation: f32 -> bf16
                        nc.scalar.copy(sqb[:, bass.ts(h, 2 * NPTS)], psq[:])
                        if h == 0:
                            nc.vector.tensor_tensor(
                                m01[:], sqb[:, 0:NPTS], sqb[:, NPTS : 2 * NPTS],
                                op=AluOp.min,
                            )
                        else:
                            nc.vector.tensor_tensor(
                                m23[:], sqb[:, 2 * NPTS : 3 * NPTS],
                                sqb[:, 3 * NPTS : 4 * NPTS], op=AluOp.min,
                            )
                    # min over m: one 4x-mode reduce across all 4 chunks
                    sqb3 = sqb[:].rearrange("p (c m) -> p c m", c=NCHUNK)
                    nc.vector.tensor_reduce(
                        mins[:, 4 * b : 4 * b + 4], sqb3,
                        axis=mybir.AxisListType.X, op=AluOp.min,
                    )
                    # cross-chunk elementwise min (GPSIMD has no min ucode)
                    nc.vector.tensor_tensor(acc[:], m01[:], m23[:], op=AluOp.min)
                    # min over n: PE transpose + free-dim reduce
                    pt = pstp.tile([128, NPTS], BF16, tag="pt", bufs=1)
                    for j in range(NCHUNK):
                        nc.tensor.transpose(
                            pt[:, bass.ts(j, 128)], acc[:, bass.ts(j, 128)],
                            ident[:],
                        )
                    pt3 = pt[:].rearrange("p (c f) -> p c f", c=NCHUNK)
                    nc.vector.tensor_reduce(
                        mins[:, 32 + 4 * b : 32 + 4 * b + 4], pt3,
                        axis=mybir.AxisListType.X, op=AluOp.min,
                    )

                nc.vector.tensor_tensor(macc[:], macc[:], mins[:], op=AluOp.min)

            # -------- epilogue: clamp, sqrt, sum all --------------------
            # quartic cancellation can leave tiny negative sq: clamp first
            mclamp = finp.tile([128, 64], F32, tag="mclamp")
            nc.vector.tensor_scalar_max(mclamp[:], macc[:], 0.0)
            smins = finp.tile([128, 64], F32, tag="smins")
            nc.scalar.activation(smins[:], mclamp[:], ACT.Sqrt, bias=eps_t[:])
            colsum = finp.tile([128, 1], F32, tag="colsum")
            nc.vector.tensor_reduce(
                colsum[:], smins[:], axis=mybir.AxisListType.X, op=AluOp.add
            )
            ps_out = pstp.tile([1, 1], F32, tag="ps_out", bufs=1)
            nc.tensor.matmul(ps_out[:], ones_t[:], colsum[:])
            out_sb = finp.tile([1, 1], F32, tag="out_sb")
            nc.vector.tensor_copy(out_sb[:], ps_out[:])
            nc.sync.dma_start(out_d[:], out_sb[:])

    nc.compile()
    return nc


_NC_CACHE = {}


def _get_nc():
    if "nc" not in _NC_CACHE:
        _NC_CACHE["nc"] = build_nc()
    return _NC_CACHE["nc"]


# ---------------- host-side feature construction ----------------------

_LAM = np.array([1.0, -1.0, -1.0, -1.0], dtype=np.float32)


def _rnd11(a):
    """Round fp32 to 11 explicit mantissa bits (nearest), so PE f32r
    truncation is a no-op."""
    a = np.ascontiguousarray(a, np.float32).view(np.uint32).astype(np.uint64)
    add = np.uint64(1 << 11)
    mask = np.uint64(0xFFFFF000)
    return ((a + add) & mask).astype(np.uint32).view(np.float32)


def _pairfeat(a, fac):
    """All i<=j products of feature cols (batched); off-diag scaled by fac.
    a: (B, N, k) -> (B, N, k*(k+1)/2)."""
    k = a.shape[-1]
    iu, ju = np.triu_indices(k)
    f = np.where(iu < ju, fac, 1.0).astype(np.float32)
    return a[..., iu] * a[..., ju] * f


def _features(p, q):
    """p, q (2, B, 512, 4) -> quartic factors K, L (B, 512, 76) fp32."""
    pre, pim, qre, qim = p[0], p[1], q[0], q[1]
    B, N = pre.shape[0], pre.shape[1]
    ones = np.ones((B, N, 1), np.float32)
    alpha = (pre * pre * _LAM).sum(-1, keepdims=True)
    h = (pre * pim * _LAM).sum(-1, keepdims=True)
    x = np.concatenate([alpha, ones, -2.0 * _LAM * pre], -1)
    s = np.concatenate([2.0 * h, ones, -2.0 * _LAM * pre, -2.0 * _LAM * pim], -1)
    gamma = (qre * qre * _LAM).sum(-1, keepdims=True)
    g = (qre * qim * _LAM).sum(-1, keepdims=True)
    y = np.concatenate([ones, gamma, qre], -1)
    t = np.concatenate([ones, 2.0 * g, qim, qre], -1)
    K = np.concatenate([_pairfeat(x, 2.0), _pairfeat(s, 2.0)], -1)
    L = np.concatenate([_pairfeat(y, 1.0), _pairfeat(t, 1.0)], -1)
    return K, L


def _dekker_sides(K, L):
    """Per batch: split every feature 3-term Dekker.  K, L (512, 76) ->
    MM1 operands (512, 128) 11-bit fp32, MM2 operands (512, 100) bf16."""
    import ml_dtypes

    Kh = _rnd11(K)
    Lh = _rnd11(L)
    Klo = K - Kh
    Llo = L - Lh
    corrK = np.concatenate([Kh, Klo], 1)    # T2 = Kh*Llo, T3 = Klo*Lh
    corrL = np.concatenate([Llo, Lh], 1)
    mag = np.abs(corrK).max(0) * np.abs(corrL).max(0)
    order = np.argsort(-mag)
    top, rest = order[:52], order[52:]
    k1 = np.concatenate([Kh, _rnd11(corrK[:, top])], 1)
    l1 = np.concatenate([Lh, _rnd11(corrL[:, top])], 1)
    k2 = corrK[:, rest].astype(ml_dtypes.bfloat16)
    l2 = corrL[:, rest].astype(ml_dtypes.bfloat16)
    return k1, l1, k2, l2


def make_in_maps(p, q, v_dtype_key="bf16"):
    """Host-side shard + quartic feature layout."""
    import ml_dtypes

    p = np.asarray(p, dtype=np.float32)
    q = np.asarray(q, dtype=np.float32)
    Kall, Lall = _features(p, q)   # (64, 512, 76)
    ident = np.eye(128, dtype=ml_dtypes.bfloat16)
    NW = B_PER_CORE * NPTS
    in_maps = []
    for c in range(N_CORES):
        kf1 = np.zeros((K1, NW), np.float32)
        lf1 = np.zeros((K1, NW), np.float32)
        kf2 = np.zeros((K2, NW), ml_dtypes.bfloat16)
        lf2 = np.zeros((K2, NW), ml_dtypes.bfloat16)
        for b in range(B_PER_CORE):
            gb = c * B_PER_CORE + b
            a1, b1, a2, b2 = _dekker_sides(Kall[gb], Lall[gb])
            sl = slice(b * NPTS, (b + 1) * NPTS)
            kf1[:, sl] = a1.T
            lf1[:, sl] = b1.T
            kf2[:, sl] = a2.T
            lf2[:, sl] = b2.T
        in_maps.append(
            {"kf1": kf1, "lf1": lf1, "kf2": kf2, "lf2": lf2, "ident": ident}
        )
    return in_maps


def kernel(p, q):
    nc = _get_nc()
    in_maps = make_in_maps(p, q)
    res = run_bass_kernel_spmd(nc, in_maps, core_ids=list(range(N_CORES)))
    total = np.float64(0.0)
    for r in res.results:
        total += np.float64(r["out"][0, 0])
    return np.asarray(total, dtype=np.float32).reshape(())


# revision 7
# speedup vs baseline: 12.1121x; 9.6075x over previous
"""Chamfer loss (complex Minkowski metric) Trainium2 Bass kernel.

Full inputs p, q: (2, 64, 512, 4) fp32.  Output: scalar fp32.

Math: for each (b, n, m):
  m_real = <d_re, d_re>_L,  m_im = 2 <d_re, d_im>_L   (L = diag(1,-1,-1,-1))
  dist   = sqrt(m_real^2 + m_im^2 + eps)
  loss   = sum_bn min_m dist + sum_bm min_n dist

Key idea: sq = m_real^2 + m_im^2 is a bilinear form of rank 76 in
quartic monomial features of the two point sets, so one matmul per
(batch, n-chunk) yields sq directly in PSUM -- no ACT Square, no DVE
add.  PE f32r truncates inputs to 11 mantissa bits, which is far too
coarse for the tiny minima, so each feature f is Dekker-split:
  K_f L_f = Kh Lh + Kh Llo + Klo Lh   (+ O(2^-24) dropped)
giving 76 + 152 = 228 rank-1 terms.  Two PSUM-accumulating matmuls per
chunk: MM1 f32r K=128 (the 76 hi*hi terms + 52 largest corrections,
all pre-rounded to 11 bits so hardware truncation is a no-op), MM2
bf16 K=100 (remaining corrections; they are 2^-12-scale, so bf16 is
plenty).  Measured end-to-end rel err ~1e-4.

Device per batch: 8 matmuls -> PSUM f32; ACT copies PSUM -> SBUF bf16
(the only full-matrix evacuation); DVE reduce = min over m; DVE
elementwise-min chain over the 4 n-chunks; PE transpose + DVE reduce =
min over n; sqrt/sum epilogue to one scalar per core.

Sharding: pure data parallel over batch (8 per core); host sums the 8
per-core partials.
"""

import os

import numpy as np

import concourse.bacc as bacc
import concourse.bass as bass
import concourse.mybir as mybir
import concourse.tile as tile
from concourse.bass_utils import run_bass_kernel_spmd

AluOp = mybir.AluOpType
F32R = mybir.dt.float32r
ACT = mybir.ActivationFunctionType
F32 = mybir.dt.float32
BF16 = mybir.dt.bfloat16

N_CORES = 8
B_PER_CORE = 8
NPTS = 512
NCHUNK = 4
K1 = 128            # MM1 rows: 76 hi*hi + 52 largest corrections (f32r)
K2 = 100            # MM2 rows: remaining Dekker corrections (bf16)
BIG = 3.0e38
EPS = 1e-12


def build_nc(v_dtype=BF16, reps=1):
    """Build the per-core SPMD Bass module.

    reps > 1 repeats the whole compute loop (same data, idempotent
    results) so wall-clock slope over reps isolates per-iteration device
    time from dispatch overhead.
    """
    nc = bacc.Bacc("TRN2", target_bir_lowering=False, debug=False)

    NW = B_PER_CORE * NPTS
    kf1_d = nc.dram_tensor("kf1", [K1, NW], F32, kind="ExternalInput")
    lf1_d = nc.dram_tensor("lf1", [K1, NW], F32, kind="ExternalInput")
    kf2_d = nc.dram_tensor("kf2", [K2, NW], BF16, kind="ExternalInput")
    lf2_d = nc.dram_tensor("lf2", [K2, NW], BF16, kind="ExternalInput")
    ident_d = nc.dram_tensor("ident", [128, 128], BF16, kind="ExternalInput")
    out_d = nc.dram_tensor("out", [1, 1], F32, kind="ExternalOutput")

    with tile.TileContext(nc) as tc:
        with (
            tc.tile_pool(name="const", bufs=1) as constp,
            tc.tile_pool(name="prep", bufs=1) as prep,
            tc.tile_pool(name="wts", bufs=1) as wp,
            tc.tile_pool(name="sq", bufs=3) as sqp,
            tc.tile_pool(name="red", bufs=2) as redp,
            tc.tile_pool(name="fin", bufs=1) as finp,
            tc.tile_pool(name="ps", bufs=3, space=bass.MemorySpace.PSUM) as psp,
            tc.tile_pool(name="pst", bufs=1, space=bass.MemorySpace.PSUM) as pstp,
        ):
            # -------- one-time input staging (outside the rep loop) -----
            # DMA hop DRAM -> staging -> operand tile: the BIR verifier
            # requires f32r matmul operands to come from an SBUF source.
            stg_k = prep.tile([K1, NW], F32, tag="stg_k")
            stg_l = prep.tile([K1, NW], F32, tag="stg_l")
            nc.sync.dma_start(stg_k[:], kf1_d[:])
            nc.sync.dma_start(stg_l[:], lf1_d[:])
            kf1 = wp.tile([K1, NW], F32, tag="kf1")
            lf1 = wp.tile([K1, NW], F32, tag="lf1")
            nc.sync.dma_start(kf1[:], stg_k[:])
            nc.sync.dma_start(lf1[:], stg_l[:])
            kf2 = wp.tile([K2, NW], BF16, tag="kf2")
            lf2 = wp.tile([K2, NW], BF16, tag="lf2")
            nc.sync.dma_start(kf2[:], kf2_d[:])
            nc.sync.dma_start(lf2[:], lf2_d[:])
            ident = constp.tile([128, 128], BF16, tag="ident")
            nc.scalar.dma_start(ident[:], ident_d[:])
            eps_t = constp.tile([128, 1], F32, tag="eps")
            nc.gpsimd.memset(eps_t[:], EPS)
            ones_t = constp.tile([128, 1], F32, tag="ones")
            nc.gpsimd.memset(ones_t[:], 1.0)

            # mins cols 0:32  -> min over m (4 n-chunks per batch)
            # mins cols 32:64 -> min over n (4 m-blocks per batch)
            macc = finp.tile([128, 64], BF16, tag="macc")
            nc.gpsimd.memset(macc[:], BIG)

            for rep in range(reps):
                mins = finp.tile([128, 64], BF16, tag="mins", bufs=2,
                                 name=f"mins{rep}")
                for b in range(B_PER_CORE):
                    ob = b * NPTS
                    sqb = sqp.tile([128, NCHUNK * NPTS], BF16, tag="sqb")
                    for h in range(2):
                        psq = psp.tile([128, 2 * NPTS], F32, tag="psq")
                        for j in range(2):
                            c = 2 * h + j
                            cs = slice(ob + 128 * c, ob + 128 * c + 128)
                            nc.tensor.matmul(
                                psq[:, bass.ts(j, NPTS)],
                                kf1[:, cs].bitcast(F32R),
                                lf1[:, ob : ob + NPTS].bitcast(F32R),
                                start=True, stop=False,
                            )
                            nc.tensor.matmul(
                                psq[:, bass.ts(j, NPTS)],
                                kf2[:, cs],
                                lf2[:, ob : ob + NPTS],
                                start=False, stop=True,
                            )
                        # the ONLY full-matrix PSUM evacuation: f32 -> bf16
                        nc.scalar.copy(sqb[:, bass.ts(h, 2 * NPTS)], psq[:])
                    # tensor_reduce runs 1x; TT-min runs 2x on bf16, so all
                    # mins are built from pairwise TTs plus tiny tail reduces.
                    # rowmin (min over m per n): in-chunk halving L1..L4
                    r1 = redp.tile([128, 4 * 256], BF16, tag="r1")
                    v = sqb[:].rearrange("p (c m) -> p c m", c=NCHUNK)
                    nc.vector.tensor_tensor(
                        r1[:].rearrange("p (c m) -> p c m", c=NCHUNK),
                        v[:, :, 0:256], v[:, :, 256:512], op=AluOp.min)
                    r2 = redp.tile([128, 4 * 128], BF16, tag="r2")
                    v = r1[:].rearrange("p (c m) -> p c m", c=NCHUNK)
                    nc.vector.tensor_tensor(
                        r2[:].rearrange("p (c m) -> p c m", c=NCHUNK),
                        v[:, :, 0:128], v[:, :, 128:256], op=AluOp.min)
                    r3 = redp.tile([128, 4 * 64], BF16, tag="r3")
                    v = r2[:].rearrange("p (c m) -> p c m", c=NCHUNK)
                    nc.vector.tensor_tensor(
                        r3[:].rearrange("p (c m) -> p c m", c=NCHUNK),
                        v[:, :, 0:64], v[:, :, 64:128], op=AluOp.min)
                    r4 = redp.tile([128, 4 * 32], BF16, tag="r4")
                    v = r3[:].rearrange("p (c m) -> p c m", c=NCHUNK)
                    nc.vector.tensor_tensor(
                        r4[:].rearrange("p (c m) -> p c m", c=NCHUNK),
                        v[:, :, 0:32], v[:, :, 32:64], op=AluOp.min)
                    nc.vector.tensor_reduce(
                        mins[:, 4 * b : 4 * b + 4],
                        r4[:].rearrange("p (c m) -> p c m", c=NCHUNK),
                        axis=mybir.AxisListType.X, op=AluOp.min,
                    )
                    # colmin chain: 2 cross-chunk TTs -> acc = min over chunks
                    ch1 = redp.tile([128, 2 * NPTS], BF16, tag="ch1")
                    nc.vector.tensor_tensor(
                        ch1[:], sqb[:, 0 : 2 * NPTS], sqb[:, 2 * NPTS : 4 * NPTS],
                        op=AluOp.min)
                    acc = redp.tile([128, NPTS], BF16, tag="acc")
                    nc.vector.tensor_tensor(
                        acc[:], ch1[:, 0:NPTS], ch1[:, NPTS : 2 * NPTS],
                        op=AluOp.min)
                    # min over n: PE transpose + free-dim reduce
                    pt = pstp.tile([128, NPTS], BF16, tag="pt", bufs=1)
                    for j in range(NCHUNK):
                        nc.tensor.transpose(
                            pt[:, bass.ts(j, 128)], acc[:, bass.ts(j, 128)],
                            ident[:],
                        )
                    pt3 = pt[:].rearrange("p (c f) -> p c f", c=NCHUNK)
                    nc.vector.tensor_reduce(
                        mins[:, 32 + 4 * b : 32 + 4 * b + 4], pt3,
                        axis=mybir.AxisListType.X, op=AluOp.min,
                    )

                nc.vector.tensor_tensor(macc[:], macc[:], mins[:], op=AluOp.min)

            # -------- epilogue: clamp, sqrt, sum all --------------------
            # quartic cancellation can leave tiny negative sq: clamp first
            mclamp = finp.tile([128, 64], F32, tag="mclamp")
            nc.vector.tensor_scalar_max(mclamp[:], macc[:], 0.0)
            smins = finp.tile([128, 64], F32, tag="smins")
            nc.scalar.activation(smins[:], mclamp[:], ACT.Sqrt, bias=eps_t[:])
            colsum = finp.tile([128, 1], F32, tag="colsum")
            nc.vector.tensor_reduce(
                colsum[:], smins[:], axis=mybir.AxisListType.X, op=AluOp.add
            )
            ps_out = pstp.tile([1, 1], F32, tag="ps_out", bufs=1)
            nc.tensor.matmul(ps_out[:], ones_t[:], colsum[:])
            out_sb = finp.tile([1, 1], F32, tag="out_sb")
            nc.vector.tensor_copy(out_sb[:], ps_out[:])
            nc.sync.dma_start(out_d[:], out_sb[:])

    nc.compile()
    return nc


_NC_CACHE = {}


def _get_nc():
    if "nc" not in _NC_CACHE:
        _NC_CACHE["nc"] = build_nc()
    return _NC_CACHE["nc"]


# ---------------- host-side feature construction ----------------------

_LAM = np.array([1.0, -1.0, -1.0, -1.0], dtype=np.float32)


def _rnd11(a):
    """Round fp32 to 11 explicit mantissa bits (nearest), so PE f32r
    truncation is a no-op."""
    a = np.ascontiguousarray(a, np.float32).view(np.uint32).astype(np.uint64)
    add = np.uint64(1 << 11)
    mask = np.uint64(0xFFFFF000)
    return ((a + add) & mask).astype(np.uint32).view(np.float32)


def _pairfeat(a, fac):
    """All i<=j products of feature cols (batched); off-diag scaled by fac.
    a: (B, N, k) -> (B, N, k*(k+1)/2)."""
    k = a.shape[-1]
    iu, ju = np.triu_indices(k)
    f = np.where(iu < ju, fac, 1.0).astype(np.float32)
    return a[..., iu] * a[..., ju] * f


def _features(p, q):
    """p, q (2, B, 512, 4) -> quartic factors K, L (B, 512, 76) fp32."""
    pre, pim, qre, qim = p[0], p[1], q[0], q[1]
    B, N = pre.shape[0], pre.shape[1]
    ones = np.ones((B, N, 1), np.float32)
    alpha = (pre * pre * _LAM).sum(-1, keepdims=True)
    h = (pre * pim * _LAM).sum(-1, keepdims=True)
    x = np.concatenate([alpha, ones, -2.0 * _LAM * pre], -1)
    s = np.concatenate([2.0 * h, ones, -2.0 * _LAM * pre, -2.0 * _LAM * pim], -1)
    gamma = (qre * qre * _LAM).sum(-1, keepdims=True)
    g = (qre * qim * _LAM).sum(-1, keepdims=True)
    y = np.concatenate([ones, gamma, qre], -1)
    t = np.concatenate([ones, 2.0 * g, qim, qre], -1)
    K = np.concatenate([_pairfeat(x, 2.0), _pairfeat(s, 2.0)], -1)
    L = np.concatenate([_pairfeat(y, 1.0), _pairfeat(t, 1.0)], -1)
    return K, L


def _dekker_sides(K, L):
    """Per batch: split every feature 3-term Dekker.  K, L (512, 76) ->
    MM1 operands (512, 128) 11-bit fp32, MM2 operands (512, 100) bf16."""
    import ml_dtypes

    Kh = _rnd11(K)
    Lh = _rnd11(L)
    Klo = K - Kh
    Llo = L - Lh
    corrK = np.concatenate([Kh, Klo], 1)    # T2 = Kh*Llo, T3 = Klo*Lh
    corrL = np.concatenate([Llo, Lh], 1)
    mag = np.abs(corrK).max(0) * np.abs(corrL).max(0)
    order = np.argsort(-mag)
    top, rest = order[:52], order[52:]
    k1 = np.concatenate([Kh, _rnd11(corrK[:, top])], 1)
    l1 = np.concatenate([Lh, _rnd11(corrL[:, top])], 1)
    k2 = corrK[:, rest].astype(ml_dtypes.bfloat16)
    l2 = corrL[:, rest].astype(ml_dtypes.bfloat16)
    return k1, l1, k2, l2


def make_in_maps(p, q, v_dtype_key="bf16"):
    """Host-side shard + quartic feature layout."""
    import ml_dtypes

    p = np.asarray(p, dtype=np.float32)
    q = np.asarray(q, dtype=np.float32)
    Kall, Lall = _features(p, q)   # (64, 512, 76)
    ident = np.eye(128, dtype=ml_dtypes.bfloat16)
    NW = B_PER_CORE * NPTS
    in_maps = []
    for c in range(N_CORES):
        kf1 = np.zeros((K1, NW), np.float32)
        lf1 = np.zeros((K1, NW), np.float32)
        kf2 = np.zeros((K2, NW), ml_dtypes.bfloat16)
        lf2 = np.zeros((K2, NW), ml_dtypes.bfloat16)
        for b in range(B_PER_CORE):
            gb = c * B_PER_CORE + b
            a1, b1, a2, b2 = _dekker_sides(Kall[gb], Lall[gb])
            sl = slice(b * NPTS, (b + 1) * NPTS)
            kf1[:, sl] = a1.T
            lf1[:, sl] = b1.T
            kf2[:, sl] = a2.T
            lf2[:, sl] = b2.T
        in_maps.append(
            {"kf1": kf1, "lf1": lf1, "kf2": kf2, "lf2": lf2, "ident": ident}
        )
    return in_maps


def kernel(p, q):
    nc = _get_nc()
    in_maps = make_in_maps(p, q)
    res = run_bass_kernel_spmd(nc, in_maps, core_ids=list(range(N_CORES)))
    total = np.float64(0.0)
    for r in res.results:
        total += np.float64(r["out"][0, 0])
    return np.asarray(total, dtype=np.float32).reshape(())
